# revision 1
# baseline (speedup 1.0000x reference)
"""Multi-Head Latent Attention (MLA) forward on 8 Trainium2 NeuronCores.

Problem shapes (hardcoded, self-contained):
  B=2, T=2048, D=2048, H=16, DH=128, DKV=512, DQ=1024, DR=64, fp32.

Sharding: core ci = b*4 + hg  (b in {0,1}, hg in {0..3}); each core owns one
batch element and 4 heads.  Up-projection weights sharded over heads; the
final W_O matmul is input-dim sharded, so each core emits a partial (D,T)
output which the host sums over the 4 head-group cores per batch.

Device dataflow is "feature-major": every activation is stored transposed
(features on partitions, tokens on the free dim), so every matmul contraction
lands on the partition dim with zero device-side transposes of x.  Attention
scores are computed transposed (S.T: keys on partitions, queries free);
softmax denominators come from a ones-column appended to V (the PV matmul
then yields row sums for free), and normalization happens on the small
attention output rather than on the probability matrix.

Matmuls run in float32r (full-rate fp32 mode).  The BIR verifier requires
every f32r matmul operand to be produced by an instruction that rounds to
f32r, so activation producers write f32r directly and DMA-fed tensors are
staged per-chunk through a rounding copy.

Phases (single NEFF):
  A: cQ.T = norm(x@W_DQ).T, cKV.T = norm(x@W_DKV).T, kr.T = (x@W_KR).T
     (rmsnorm over partition dim via ones-matmul sumsq + PE-broadcast)
  B: q.T/qr.T/k.T (feature-major), v (token-major, bf16, +ones col), RoPE
  C: per (head, qtile): S.T matmuls -> causal mask -> exp (ACT, bf16 out)
     -> PV matmuls (bf16) -> normalize by ones-column rowsum -> PE-transpose
     into feature-major attention output
  D: final.T = W_O_shard.T @ attnout.T  -> DRAM (D, T)
"""

import math

import numpy as np

B, T, D = 2, 2048, 2048
H, DH = 16, 128
DKV, DQ, DR = 512, 1024, 64
ROPE_BASE = 500000.0
EPS = 1e-6
SCALE = 1.0 / math.sqrt(DH + DR)

HL = 4            # heads per core
NCORES = 8
TW = 512          # token tile width (free dim)
NT = T // TW      # 4 token tiles
NKC = D // 128    # 16 contraction chunks over D
NEG = -1.0e30

_CACHE: dict = {}
LAST_EXEC_NS = None


def _build():
    from contextlib import ExitStack

    import concourse.mybir as mybir
    import concourse.tile as tile
    from concourse.bacc import Bacc
    from concourse.masks import make_identity

    f32 = mybir.dt.float32
    f32r = mybir.dt.float32r
    bf16 = mybir.dt.bfloat16
    AF = mybir.ActivationFunctionType

    nc = Bacc("TRN2")

    xT_d = nc.dram_tensor("xt", (D, T), f32, kind="ExternalInput")
    wdq_d = nc.dram_tensor("wdq", (D, DQ), f32, kind="ExternalInput")
    wkvr_d = nc.dram_tensor("wkvr", (D, 640), f32, kind="ExternalInput")
    wuq_d = nc.dram_tensor("wuq", (DQ, HL * DH), f32, kind="ExternalInput")
    wqr_d = nc.dram_tensor("wqr", (DQ, HL * DR), f32, kind="ExternalInput")
    wuk_d = nc.dram_tensor("wuk", (DKV, HL * DH), f32, kind="ExternalInput")
    wuv_d = nc.dram_tensor("wuv", (DKV, HL * DH), f32, kind="ExternalInput")
    wo_d = nc.dram_tensor("wo", (HL * DH, D), f32, kind="ExternalInput")
    cos_d = nc.dram_tensor("costab", (128, T), f32, kind="ExternalInput")
    sin_d = nc.dram_tensor("sintab", (128, T), f32, kind="ExternalInput")
    out_d = nc.dram_tensor("final_t", (D, T), f32, kind="ExternalOutput")

    with tile.TileContext(nc) as tc, ExitStack() as ctx:
        persist = ctx.enter_context(tc.tile_pool(name="persist", bufs=1))
        dramp = ctx.enter_context(tc.tile_pool(name="dram", bufs=1, space="DRAM"))

        # --- constants ---
        ones_raw = persist.tile([128, 128], f32, tag="ones_raw")
        nc.gpsimd.memset(ones_raw, 1.0)
        ones_sb = persist.tile([128, 128], f32r, tag="ones")
        nc.vector.tensor_copy(ones_sb, ones_raw)
        ident_sb = persist.tile([128, 128], f32, tag="ident")
        make_identity(nc, ident_sb)
        eps_sb = persist.tile([1, 1], f32, tag="eps")
        nc.vector.memset(eps_sb, EPS)
        # --- persistent activations (phase A outputs only) ---
        kr_raw = persist.tile([128, T], f32, tag="kr_raw")
        kr_rope = persist.tile([128, T], f32r, tag="kr_rope")

        cq_dram = dramp.tile([DQ, T], f32r, tag="cq")
        ckv_dram = dramp.tile([DKV, T], f32r, tag="ckv")

        # DMA f32 bits straight into the f32r tile, then round in place
        # (the verifier keys on the last writer, which is the rounding copy)
        def load_rounded(dst_r, src_view, nchunk):
            # dst_r: f32r tile [128, nchunk, w]; src_view: dram AP same shape
            nc.sync.dma_start(out=dst_r, in_=src_view.bitcast(f32r))
            for c in range(nchunk):
                nc.vector.tensor_copy(
                    dst_r[:, c, :], dst_r[:, c, :].bitcast(f32))

        # ================= Phase A =================
        with tc.tile_pool(name="wA", bufs=1) as wA, \
             tc.tile_pool(name="xA", bufs=2) as xA, \
             tc.tile_pool(name="rawA", bufs=2) as rawA, \
             tc.tile_pool(name="sqA", bufs=2) as sqA, \
             tc.tile_pool(name="nrmA", bufs=2) as nrmA, \
             tc.tile_pool(name="psAmm", bufs=4, space="PSUM") as psAmm, \
             tc.tile_pool(name="psAsum", bufs=2, space="PSUM") as psAsum, \
             tc.tile_pool(name="psAbc", bufs=2, space="PSUM") as psAbc:

            for g, (w_d, mtot, n_norm, ndiv, o_dram) in enumerate([
                (wdq_d, DQ, DQ // 128, DQ, cq_dram),
                (wkvr_d, 640, DKV // 128, DKV, ckv_dram),
            ]):
                nmch = mtot // 128
                w_sb = wA.tile([128, NKC, mtot], f32r, tag="wA")
                load_rounded(w_sb, w_d.rearrange("(c p) m -> p c m", p=128),
                             NKC)
                for tt in range(NT):
                    ts = slice(tt * TW, (tt + 1) * TW)
                    x_sb = xA.tile([128, NKC, TW], f32r, tag="xA")
                    load_rounded(
                        x_sb, xT_d[:, ts].rearrange("(c p) t -> p c t", p=128),
                        NKC)
                    raw_sb = rawA.tile([128, nmch, TW], f32, tag="rawA")
                    sum_ps = psAsum.tile([1, TW], f32, tag="psAsum")
                    for mc in range(nmch):
                        mm_ps = psAmm.tile([128, TW], f32, tag="psAmm")
                        for kc in range(NKC):
                            nc.tensor.matmul(
                                mm_ps,
                                lhsT=w_sb[:, kc, mc * 128:(mc + 1) * 128],
                                rhs=x_sb[:, kc, :],
                                start=(kc == 0), stop=(kc == NKC - 1))
                        nc.scalar.copy(raw_sb[:, mc, :], mm_ps)
                        if mc < n_norm:
                            sq_sb = sqA.tile([128, TW], f32r, tag="sqA")
                            nc.scalar.square(sq_sb, mm_ps)
                            nc.tensor.matmul(
                                sum_ps, lhsT=ones_sb[:, 0:1], rhs=sq_sb,
                                start=(mc == 0), stop=(mc == n_norm - 1))
                    # n(t) = 1/sqrt(mean + eps), broadcast over partitions
                    nrm_f = nrmA.tile([1, TW], f32, tag="nrmA")
                    nc.scalar.activation(
                        nrm_f, sum_ps, func=AF.Sqrt, bias=eps_sb,
                        scale=1.0 / ndiv)
                    nc.vector.reciprocal(nrm_f, nrm_f)
                    nrm_r = nrmA.tile([1, TW], f32r, tag="nrmAr")
                    nc.vector.tensor_copy(nrm_r, nrm_f)
                    bc_ps = psAbc.tile([128, TW], f32, tag="psAbc")
                    nc.tensor.matmul(
                        bc_ps, lhsT=ones_sb[0:1, :], rhs=nrm_r,
                        start=True, stop=True)
                    for mc in range(n_norm):
                        nc.vector.tensor_mul(
                            raw_sb[:, mc, :].bitcast(f32r),
                            raw_sb[:, mc, :], bc_ps)
                    nc.sync.dma_start(
                        out=o_dram[:, ts].rearrange("(c p) t -> p c t", p=128),
                        in_=raw_sb[:, 0:n_norm, :].bitcast(f32r))
                    if g == 1:
                        # raw chunk 4 rows 0:64 = x @ W_KR (not normed);
                        # duplicated into both partition halves so either
                        # base-0 or base-64 qr slices can pair with it
                        nc.vector.tensor_copy(
                            kr_raw[0:64, ts], raw_sb[0:64, 4, :])
                        nc.vector.tensor_copy(
                            kr_raw[64:128, ts], raw_sb[0:64, 4, :])

        # ================= Phase B =================
        def rope(dst, src, tmp1, rot_sb, cos_ap, sin_ap, nhalf):
            # dst(f32r) = src*cos + rotate_half(src)*sin ; rows = nhalf*64
            for hh in range(nhalf):
                lo = slice(hh * 64, hh * 64 + 32)
                hi = slice(hh * 64 + 32, hh * 64 + 64)
                nc.scalar.mul(rot_sb[lo, :], src[hi, :], -1.0)
                nc.scalar.copy(rot_sb[hi, :], src[lo, :])
            n = nhalf * 64
            nc.vector.tensor_mul(tmp1[0:n, :], src[0:n, :], cos_ap[0:n, :])
            nc.vector.tensor_mul(rot_sb[0:n, :], rot_sb[0:n, :], sin_ap[0:n, :])
            nc.vector.tensor_add(dst[0:n, :], tmp1[0:n, :], rot_sb[0:n, :])

        pB = ctx.enter_context(tc.tile_pool(name="persistB", bufs=1))
        qT_sb = pB.tile([128, HL, T], f32r, tag="qT")
        qrT_sb = pB.tile([128, HL // 2, T], f32r, tag="qrT")
        kT_sb = pB.tile([128, HL, T], f32r, tag="kT")
        v_sb = pB.tile([128, HL, T // 128, 132], bf16, tag="v")
        nc.vector.memset(v_sb[:, :, :, 128:129], 1.0)

        with tc.tile_pool(name="trig", bufs=1) as trig, \
             tc.tile_pool(name="wB", bufs=1) as wB, \
             tc.tile_pool(name="actB", bufs=1) as actB, \
             tc.tile_pool(name="tmpB", bufs=1) as tmpB, \
             tc.tile_pool(name="psB", bufs=4, space="PSUM") as psB:

            cos_sb = trig.tile([128, T], f32, tag="cos")
            sin_sb = trig.tile([128, T], f32, tag="sin")
            nc.sync.dma_start(out=cos_sb, in_=cos_d[:, :])
            nc.sync.dma_start(out=sin_sb, in_=sin_d[:, :])

            # ---- B1: Q path ----
            wuq_sb = wB.tile([128, DQ // 128, HL * DH], f32r, tag="wuq")
            wqr_sb = wB.tile([128, DQ // 128, HL * DR], f32r, tag="wqr")
            load_rounded(wuq_sb, wuq_d.rearrange("(c p) m -> p c m", p=128),
                         DQ // 128)
            load_rounded(wqr_sb, wqr_d.rearrange("(c p) m -> p c m", p=128),
                         DQ // 128)
            for tt in range(NT):
                ts = slice(tt * TW, (tt + 1) * TW)
                cq_sb = actB.tile([128, DQ // 128, TW], f32r, tag="cqB")
                nc.sync.dma_start(
                    out=cq_sb,
                    in_=cq_dram[:, ts].rearrange("(c p) t -> p c t", p=128))
                for h in range(HL):
                    mm_ps = psB.tile([128, TW], f32, tag="psB")
                    for kc in range(DQ // 128):
                        nc.tensor.matmul(
                            mm_ps,
                            lhsT=wuq_sb[:, kc, h * 128:(h + 1) * 128],
                            rhs=cq_sb[:, kc, :],
                            start=(kc == 0), stop=(kc == DQ // 128 - 1))
                    nc.scalar.copy(qT_sb[:, h, ts], mm_ps)
                for j in range(HL // 2):
                    mm_ps = psB.tile([128, TW], f32, tag="psB")
                    for kc in range(DQ // 128):
                        nc.tensor.matmul(
                            mm_ps,
                            lhsT=wqr_sb[:, kc, j * 128:(j + 1) * 128],
                            rhs=cq_sb[:, kc, :],
                            start=(kc == 0), stop=(kc == DQ // 128 - 1))
                    qr_tmp = tmpB.tile([128, TW], f32, tag="qrtmp")
                    nc.scalar.copy(qr_tmp, mm_ps)
                    tmp1 = tmpB.tile([128, TW], f32, tag="tmp1")
                    rot_sb = tmpB.tile([128, TW], f32, tag="rot")
                    rope(qrT_sb[:, j, ts], qr_tmp, tmp1, rot_sb,
                         cos_sb[:, ts], sin_sb[:, ts], 2)

            # ---- B2: KV path ----
            wuk_sb = wB.tile([128, DKV // 128, HL * DH], f32r, tag="wuk")
            wuv_sb = wB.tile([128, DKV // 128, HL * DH], f32r, tag="wuv")
            load_rounded(wuk_sb, wuk_d.rearrange("(c p) m -> p c m", p=128),
                         DKV // 128)
            load_rounded(wuv_sb, wuv_d.rearrange("(c p) m -> p c m", p=128),
                         DKV // 128)
            for tt in range(NT):
                ts = slice(tt * TW, (tt + 1) * TW)
                ckv_sb = actB.tile([128, DKV // 128, TW], f32r, tag="ckvB")
                nc.sync.dma_start(
                    out=ckv_sb,
                    in_=ckv_dram[:, ts].rearrange("(c p) t -> p c t", p=128))
                for h in range(HL):
                    mm_ps = psB.tile([128, TW], f32, tag="psB")
                    for kc in range(DKV // 128):
                        nc.tensor.matmul(
                            mm_ps,
                            lhsT=wuk_sb[:, kc, h * 128:(h + 1) * 128],
                            rhs=ckv_sb[:, kc, :],
                            start=(kc == 0), stop=(kc == DKV // 128 - 1))
                    nc.scalar.copy(kT_sb[:, h, ts], mm_ps)
                # v token-major (+ cast to bf16)
                for tc4 in range(TW // 128):
                    mm_ps = psB.tile([128, TW], f32, tag="psB")
                    for kc in range(DKV // 128):
                        nc.tensor.matmul(
                            mm_ps,
                            lhsT=ckv_sb[:, kc, tc4 * 128:(tc4 + 1) * 128],
                            rhs=wuv_sb[:, kc, :],
                            start=(kc == 0), stop=(kc == DKV // 128 - 1))
                    gtc = tt * (TW // 128) + tc4
                    nc.vector.tensor_copy(
                        v_sb[:, :, gtc, 0:128],
                        mm_ps.rearrange("p (h d) -> p h d", h=HL))
                # kr rope for this ttile
                tmp1 = tmpB.tile([128, TW], f32, tag="tmp1")
                rot_sb = tmpB.tile([128, TW], f32, tag="rot")
                rope(kr_rope[:, ts], kr_raw[:, ts], tmp1, rot_sb,
                     cos_sb[:, ts], sin_sb[:, ts], 2)

        # ================= Phase C =================
        pC = ctx.enter_context(tc.tile_pool(name="persistC", bufs=1))
        aoT_sb = pC.tile([128, HL, T], f32r, tag="aoT")
        # causal masks for the 4 diagonal-crossing offsets:
        # mask_j[k_p, q_f] = NEG where (q_f - k_p - 128*j) < 0 else 0
        masks_sb = pC.tile([128, 4, TW], f32, tag="masks")
        nc.gpsimd.memset(masks_sb, 0.0)
        for j in range(4):
            nc.gpsimd.affine_select(
                out=masks_sb[:, j, :],
                in_=masks_sb[:, j, :],
                compare_op=mybir.AluOpType.is_ge,
                fill=NEG,
                base=-128 * j,
                pattern=[[1, TW]],
                channel_multiplier=-1,
            )

        with tc.tile_pool(name="ptC", bufs=2) as ptC, \
             tc.tile_pool(name="noC", bufs=2) as noC, \
             tc.tile_pool(name="rC", bufs=2) as rC, \
             tc.tile_pool(name="psS", bufs=3, space="PSUM") as psS, \
             tc.tile_pool(name="psPV", bufs=4, space="PSUM") as psPV, \
             tc.tile_pool(name="psTr", bufs=1, space="PSUM") as psTr:

            for h in range(HL):
                qr_part = slice((h % 2) * 64, (h % 2) * 64 + 64)
                jj = h // 2
                for c in range(NT):
                    nkt = 4 * c + 4
                    qs = slice(c * TW, (c + 1) * TW)
                    pt_sb = ptC.tile([128, T // 128, TW], bf16, tag="ptC")
                    pv_ps = [psPV.tile([128, 132], f32, tag="psPV",
                                       name=f"pv_{h}_{c}_{i4}")
                             for i4 in range(4)]
                    for kt in range(nkt):
                        ks = slice(kt * 128, (kt + 1) * 128)
                        s_ps = psS.tile([128, TW], f32, tag="psS")
                        nc.tensor.matmul(
                            s_ps, lhsT=kT_sb[:, h, ks],
                            rhs=qT_sb[:, h, qs],
                            start=True, stop=False)
                        nc.tensor.matmul(
                            s_ps, lhsT=kr_rope[qr_part, ks],
                            rhs=qrT_sb[qr_part, jj, qs],
                            start=False, stop=True)
                        j = kt - 4 * c
                        if j >= 0:
                            nc.vector.tensor_add(s_ps, s_ps, masks_sb[:, j, :])
                        nc.scalar.activation(
                            pt_sb[:, kt, :], s_ps, func=AF.Exp, scale=SCALE)
                    for kt in range(nkt):
                        for b4 in range(4):
                            nc.tensor.matmul(
                                pv_ps[b4][:, 0:129],
                                lhsT=pt_sb[:, kt, b4 * 128:(b4 + 1) * 128],
                                rhs=v_sb[:, h, kt, 0:129],
                                start=(kt == 0), stop=(kt == nkt - 1))
                    for b4 in range(4):
                        r_sb = rC.tile([128, 1], f32, tag="rC")
                        nc.vector.reciprocal(r_sb, pv_ps[b4][:, 128:129])
                        no_sb = noC.tile([128, 128], f32, tag="noC")
                        nc.vector.tensor_scalar_mul(
                            no_sb, pv_ps[b4][:, 0:128], r_sb)
                        tr_ps = psTr.tile([128, 128], f32, tag="psTr")
                        nc.tensor.transpose(tr_ps, no_sb, ident_sb)
                        nc.vector.tensor_copy(
                            aoT_sb[:, h,
                                   c * TW + b4 * 128:c * TW + (b4 + 1) * 128],
                            tr_ps)

        # ================= Phase D =================
        with tc.tile_pool(name="wD", bufs=1) as wD, \
             tc.tile_pool(name="oD", bufs=3) as oD, \
             tc.tile_pool(name="psD", bufs=3, space="PSUM") as psD:
            wo_sb = wD.tile([128, HL, D], f32r, tag="wo")
            load_rounded(wo_sb, wo_d.rearrange("(c p) m -> p c m", p=128),
                         HL)
            for nt in range(NT):
                ns = slice(nt * TW, (nt + 1) * TW)
                for dc in range(D // 128):
                    o_ps = psD.tile([128, TW], f32, tag="psD")
                    for hc in range(HL):
                        nc.tensor.matmul(
                            o_ps,
                            lhsT=wo_sb[:, hc, dc * 128:(dc + 1) * 128],
                            rhs=aoT_sb[:, hc, ns],
                            start=(hc == 0), stop=(hc == HL - 1))
                    o_sb = oD.tile([128, TW], f32, tag="oD")
                    nc.scalar.copy(o_sb, o_ps)
                    nc.sync.dma_start(
                        out=out_d[dc * 128:(dc + 1) * 128, ns], in_=o_sb)

    nc.finalize()
    return nc


def _rope_tables():
    inv_freq = (1.0 / (ROPE_BASE ** (np.arange(0, DR, 2, dtype=np.float32) / DR))
                ).astype(np.float32)
    t = np.arange(T, dtype=np.float32)
    freqs = np.outer(t, inv_freq).astype(np.float32)        # (T, 32)
    emb = np.concatenate([freqs, freqs], axis=-1)           # (T, 64)
    cos = np.cos(emb).astype(np.float32).T                  # (64, T)
    sin = np.sin(emb).astype(np.float32).T
    cos128 = np.ascontiguousarray(np.concatenate([cos, cos], 0))  # (128, T)
    sin128 = np.ascontiguousarray(np.concatenate([sin, sin], 0))
    return cos128, sin128


def kernel(x, W_DQ, W_UQ, W_QR, W_DKV, W_UK, W_UV, W_KR, W_O,
           q_norm_w, kv_norm_w):
    global LAST_EXEC_NS
    from concourse.bass_utils import run_bass_kernel_spmd

    x = np.asarray(x, dtype=np.float32)
    W_DQ = np.asarray(W_DQ, np.float32)
    W_UQ = np.asarray(W_UQ, np.float32)
    W_QR = np.asarray(W_QR, np.float32)
    W_DKV = np.asarray(W_DKV, np.float32)
    W_UK = np.asarray(W_UK, np.float32)
    W_UV = np.asarray(W_UV, np.float32)
    W_KR = np.asarray(W_KR, np.float32)
    W_O = np.asarray(W_O, np.float32)
    q_norm_w = np.asarray(q_norm_w, np.float32)
    kv_norm_w = np.asarray(kv_norm_w, np.float32)

    # fold norm weights into the up-projections (w==1 in practice, but general)
    wuq_f = W_UQ * q_norm_w[:, None]
    wqr_f = W_QR * q_norm_w[:, None]
    wuk_f = W_UK * kv_norm_w[:, None]
    wuv_f = W_UV * kv_norm_w[:, None]

    wkvr = np.ascontiguousarray(
        np.concatenate([W_DKV, W_KR, np.zeros((D, 64), np.float32)], axis=1))
    cos128, sin128 = _rope_tables()

    wuq_h = wuq_f.reshape(DQ, H, DH)
    wqr_h = wqr_f.reshape(DQ, H, DR)
    wuk_h = wuk_f.reshape(DKV, H, DH)
    wuv_h = wuv_f.reshape(DKV, H, DH)
    wo_h = W_O.reshape(H, DH, D)

    in_maps = []
    for ci in range(NCORES):
        b, hg = divmod(ci, H // HL)
        hsl = slice(hg * HL, (hg + 1) * HL)
        in_maps.append({
            "xt": np.ascontiguousarray(x[b].T),
            "wdq": np.ascontiguousarray(W_DQ),
            "wkvr": wkvr,
            "wuq": np.ascontiguousarray(wuq_h[:, hsl].reshape(DQ, HL * DH)),
            "wqr": np.ascontiguousarray(wqr_h[:, hsl].reshape(DQ, HL * DR)),
            "wuk": np.ascontiguousarray(wuk_h[:, hsl].reshape(DKV, HL * DH)),
            "wuv": np.ascontiguousarray(wuv_h[:, hsl].reshape(DKV, HL * DH)),
            "wo": np.ascontiguousarray(wo_h[hsl].reshape(HL * DH, D)),
            "costab": cos128,
            "sintab": sin128,
        })

    if "nc" not in _CACHE:
        _CACHE["nc"] = _build()
    nc = _CACHE["nc"]

    import os as _os
    _trace = _os.environ.get("MLA_TRACE") == "1"
    res = run_bass_kernel_spmd(
        nc, in_maps, core_ids=list(range(NCORES)), trace=_trace)
    LAST_EXEC_NS = res.exec_time_ns
    outs = [res.results[ci]["final_t"] for ci in range(NCORES)]

    out = np.zeros((B, T, D), np.float32)
    for ci in range(NCORES):
        b = ci // (H // HL)
        out[b] += outs[ci].T
    return out



# revision 6
# speedup vs baseline: 1.3421x; 1.3421x over previous
"""Multi-Head Latent Attention (MLA) forward on 8 Trainium2 NeuronCores.

Problem shapes (hardcoded, self-contained):
  B=2, T=2048, D=2048, H=16, DH=128, DKV=512, DQ=1024, DR=64, fp32 I/O.

Sharding: core ci = b*4 + hg  (b in {0,1}, hg in {0..3}); each core owns one
batch element and 4 heads.  Up-projection weights sharded over heads; the
final W_O matmul is input-dim sharded, so each core emits a partial (D,T)
output which the host sums (in f32) over the 4 head-group cores per batch.

All device matmul operands are bf16 (PSUM accumulation stays f32); the host
pre-converts inputs, so there are no on-device rounding copies.  Dataflow is
feature-major (features on partitions, tokens free) so every contraction
lands on the partition dim with no transposes of activations.

Phases (single NEFF):
  A+B fused per 512-token tile: latents (cq | ckv | kr) from one x-tile pass
     (W_KR is packed twice in the last 128-column block so the duplicated
     kr halves come out of the same matmul), rmsnorm via ones-matmul sumsq
     + PE-broadcast of 1/rms, then immediately up-project to qT/qrT/kT/v
     while the latents are still in SBUF.  RoPE applied to qr/kr inline.
  C: per (head, 1024-query half): exact-width causal S^T blocks (keys on
     partitions), one static 128x128 diagonal mask, exp -> bf16 P^T, PV
     matmuls interleaved per key-block into a wide PSUM with a ones-column
     denominator, normalize + PE-transpose into feature-major aoT.
  D: final.T = W_O_shard.T @ aoT -> DRAM (D, T) bf16.
"""

import math

import numpy as np

B, T, D = 2, 2048, 2048
H, DH = 16, 128
DKV, DQ, DR = 512, 1024, 64
ROPE_BASE = 500000.0
EPS = 1e-6
SCALE = 1.0 / math.sqrt(DH + DR)

HL = 4            # heads per core
NCORES = 8
TW = 512          # token tile width for A+B
NT = T // TW      # 4 token tiles
NKC = D // 128    # 16 contraction chunks over D
MTOT = 1664       # latent columns: 1024 cq | 512 ckv | 64 kr | 64 kr (dup)
NMC = MTOT // 128  # 13 column blocks
NEG = -1.0e30

_CACHE: dict = {}
LAST_EXEC_NS = None


def _build():
    from contextlib import ExitStack

    import concourse.mybir as mybir
    import concourse.tile as tile
    from concourse.bacc import Bacc
    from concourse.masks import make_identity

    f32 = mybir.dt.float32
    bf16 = mybir.dt.bfloat16
    AF = mybir.ActivationFunctionType

    nc = Bacc("TRN2")

    xT_d = nc.dram_tensor("xt", (D, T), bf16, kind="ExternalInput")
    wall_d = nc.dram_tensor("wall", (D, MTOT), bf16, kind="ExternalInput")
    wuq_d = nc.dram_tensor("wuq", (DQ, HL * DH), bf16, kind="ExternalInput")
    wqr_d = nc.dram_tensor("wqr", (DQ, HL * DR), bf16, kind="ExternalInput")
    wuk_d = nc.dram_tensor("wuk", (DKV, HL * DH), bf16, kind="ExternalInput")
    wuv_d = nc.dram_tensor("wuv", (DKV, HL * DH), bf16, kind="ExternalInput")
    wo_d = nc.dram_tensor("wo", (HL * DH, D), bf16, kind="ExternalInput")
    cos_d = nc.dram_tensor("costab", (128, T), bf16, kind="ExternalInput")
    sin_d = nc.dram_tensor("sintab", (128, T), bf16, kind="ExternalInput")
    out_d = nc.dram_tensor("final_t", (D, T), bf16, kind="ExternalOutput")

    with tile.TileContext(nc) as tc, ExitStack() as ctx:
        persist = ctx.enter_context(tc.tile_pool(name="persist", bufs=1))

        # --- constants ---
        ones_sb = persist.tile([128, 128], bf16, tag="ones")
        nc.gpsimd.memset(ones_sb, 1.0)
        ident_sb = persist.tile([128, 128], f32, tag="identf")
        make_identity(nc, ident_sb)
        ident_bf = persist.tile([128, 128], bf16, tag="identbf")
        nc.vector.tensor_copy(ident_bf, ident_sb)
        eps_sb = persist.tile([1, 1], f32, tag="eps")
        nc.vector.memset(eps_sb, EPS)

        # --- persistent activations (consumed by phase C/D) ---
        qT_sb = persist.tile([128, HL, T], bf16, tag="qT")
        qrT_sb = persist.tile([128, HL // 2, T], bf16, tag="qrT")
        kT_sb = persist.tile([128, HL, T], bf16, tag="kT")
        v_sb = persist.tile([128, HL, T // 128, 132], bf16, tag="v")
        nc.vector.memset(v_sb[:, :, :, 128:129], 1.0)
        kr_rope = persist.tile([128, T], bf16, tag="kr_rope")

        def rope(dst, src, tmp1, rot_sb, cos_ap, sin_ap):
            # dst = src*cos + rotate_half(src)*sin over two 64-row halves
            for hh in range(2):
                lo = slice(hh * 64, hh * 64 + 32)
                hi = slice(hh * 64 + 32, hh * 64 + 64)
                nc.scalar.mul(rot_sb[lo, :], src[hi, :], -1.0)
                nc.scalar.copy(rot_sb[hi, :], src[lo, :])
            nc.vector.tensor_mul(tmp1, src, cos_ap)
            nc.vector.tensor_mul(rot_sb, rot_sb, sin_ap)
            nc.vector.tensor_add(dst, tmp1, rot_sb)

        # ================= Phase A+B (fused per token tile) =================
        with tc.tile_pool(name="wA", bufs=1) as wA, \
             tc.tile_pool(name="wB", bufs=1) as wB, \
             tc.tile_pool(name="trig", bufs=1) as trig, \
             tc.tile_pool(name="xA", bufs=2) as xA, \
             tc.tile_pool(name="cqP", bufs=1) as cqP, \
             tc.tile_pool(name="ckvP", bufs=1) as ckvP, \
             tc.tile_pool(name="krP", bufs=1) as krP, \
             tc.tile_pool(name="sqA", bufs=2) as sqA, \
             tc.tile_pool(name="nrmA", bufs=1) as nrmA, \
             tc.tile_pool(name="tmpB", bufs=1) as tmpB, \
             tc.tile_pool(name="psMM", bufs=4, space="PSUM") as psMM, \
             tc.tile_pool(name="psSum", bufs=2, space="PSUM") as psSum, \
             tc.tile_pool(name="psBC", bufs=2, space="PSUM") as psBC:

            # weight / trig loads (per-chunk DMAs so compute starts early)
            wall_sb = wA.tile([128, NKC, MTOT], bf16, tag="wall")
            wall_v = wall_d.rearrange("(c p) m -> p c m", p=128)
            for kc in range(NKC):
                nc.sync.dma_start(out=wall_sb[:, kc, :], in_=wall_v[:, kc, :])
            wuq_sb = wB.tile([128, DQ // 128, HL * DH], bf16, tag="wuq")
            wqr_sb = wB.tile([128, DQ // 128, HL * DR], bf16, tag="wqr")
            wuk_sb = wB.tile([128, DKV // 128, HL * DH], bf16, tag="wuk")
            wuv_sb = wB.tile([128, DKV // 128, HL * DH], bf16, tag="wuv")
            nc.sync.dma_start(
                out=wuq_sb, in_=wuq_d.rearrange("(c p) m -> p c m", p=128))
            nc.sync.dma_start(
                out=wqr_sb, in_=wqr_d.rearrange("(c p) m -> p c m", p=128))
            nc.sync.dma_start(
                out=wuk_sb, in_=wuk_d.rearrange("(c p) m -> p c m", p=128))
            nc.sync.dma_start(
                out=wuv_sb, in_=wuv_d.rearrange("(c p) m -> p c m", p=128))
            cos_sb = trig.tile([128, T], bf16, tag="cos")
            sin_sb = trig.tile([128, T], bf16, tag="sin")
            nc.sync.dma_start(out=cos_sb, in_=cos_d[:, :])
            nc.sync.dma_start(out=sin_sb, in_=sin_d[:, :])

            for tt in range(NT):
                ts = slice(tt * TW, (tt + 1) * TW)
                x_sb = xA.tile([128, NKC, TW], bf16, tag="xA")
                nc.sync.dma_start(
                    out=x_sb,
                    in_=xT_d[:, ts].rearrange("(c p) t -> p c t", p=128))

                cq_sb = cqP.tile([128, DQ // 128, TW], bf16, tag="cq")
                ckv_sb = ckvP.tile([128, DKV // 128, TW], bf16, tag="ckv")
                kr_sb = krP.tile([128, TW], bf16, tag="kr")

                # latent matmuls + rmsnorm statistics
                sum_cq = psSum.tile([1, TW], f32, tag="psSum")
                sum_kv = psSum.tile([1, TW], f32, tag="psSum")
                for mc in range(NMC):
                    mm_ps = psMM.tile([128, TW], f32, tag="psMM")
                    for kc in range(NKC):
                        nc.tensor.matmul(
                            mm_ps,
                            lhsT=wall_sb[:, kc, mc * 128:(mc + 1) * 128],
                            rhs=x_sb[:, kc, :],
                            start=(kc == 0), stop=(kc == NKC - 1))
                    if mc < 8:
                        dst = cq_sb[:, mc, :]
                    elif mc < 12:
                        dst = ckv_sb[:, mc - 8, :]
                    else:
                        dst = kr_sb
                    nc.scalar.copy(dst, mm_ps)
                    if mc < 12:
                        sq_sb = sqA.tile([128, TW], bf16, tag="sqA")
                        nc.scalar.square(sq_sb, mm_ps)
                        grp = sum_cq if mc < 8 else sum_kv
                        nc.tensor.matmul(
                            grp, lhsT=ones_sb[:, 0:1], rhs=sq_sb,
                            start=(mc % 8 == 0), stop=(mc in (7, 11)))

                # 1/rms factors, broadcast to 128 partitions via PE
                for grp, ndiv, nch, act in (
                        (sum_cq, DQ, 8, cq_sb), (sum_kv, DKV, 4, ckv_sb)):
                    nrm_f = nrmA.tile([1, TW], f32, tag="nrmA")
                    nc.scalar.activation(
                        nrm_f, grp, func=AF.Sqrt, bias=eps_sb, scale=1.0 / ndiv)
                    nc.vector.reciprocal(nrm_f, nrm_f)
                    nrm_bf = nrmA.tile([1, TW], bf16, tag="nrmBf")
                    nc.vector.tensor_copy(nrm_bf, nrm_f)
                    bc_ps = psBC.tile([128, TW], f32, tag="psBC")
                    nc.tensor.matmul(
                        bc_ps, lhsT=ones_sb[0:1, :], rhs=nrm_bf,
                        start=True, stop=True)
                    bc_sb = nrmA.tile([128, TW], bf16, tag="bcSb")
                    nc.vector.tensor_copy(bc_sb, bc_ps)
                    for mc in range(nch):
                        nc.vector.tensor_mul(
                            act[:, mc, :], act[:, mc, :], bc_sb)

                # ---- B: up-projections for this tile ----
                for h in range(HL):
                    mm_ps = psMM.tile([128, TW], f32, tag="psMM")
                    for kc in range(DQ // 128):
                        nc.tensor.matmul(
                            mm_ps,
                            lhsT=wuq_sb[:, kc, h * 128:(h + 1) * 128],
                            rhs=cq_sb[:, kc, :],
                            start=(kc == 0), stop=(kc == DQ // 128 - 1))
                    nc.scalar.copy(qT_sb[:, h, ts], mm_ps)
                for j in range(HL // 2):
                    mm_ps = psMM.tile([128, TW], f32, tag="psMM")
                    for kc in range(DQ // 128):
                        nc.tensor.matmul(
                            mm_ps,
                            lhsT=wqr_sb[:, kc, j * 128:(j + 1) * 128],
                            rhs=cq_sb[:, kc, :],
                            start=(kc == 0), stop=(kc == DQ // 128 - 1))
                    tmp1 = tmpB.tile([128, TW], f32, tag="tmp1")
                    rot_sb = tmpB.tile([128, TW], f32, tag="rot")
                    rope(qrT_sb[:, j, ts], mm_ps, tmp1, rot_sb,
                         cos_sb[:, ts], sin_sb[:, ts])
                for h in range(HL):
                    mm_ps = psMM.tile([128, TW], f32, tag="psMM")
                    for kc in range(DKV // 128):
                        nc.tensor.matmul(
                            mm_ps,
                            lhsT=wuk_sb[:, kc, h * 128:(h + 1) * 128],
                            rhs=ckv_sb[:, kc, :],
                            start=(kc == 0), stop=(kc == DKV // 128 - 1))
                    nc.scalar.copy(kT_sb[:, h, ts], mm_ps)
                for tc4 in range(TW // 128):
                    mm_ps = psMM.tile([128, TW], f32, tag="psMM")
                    for kc in range(DKV // 128):
                        nc.tensor.matmul(
                            mm_ps,
                            lhsT=ckv_sb[:, kc, tc4 * 128:(tc4 + 1) * 128],
                            rhs=wuv_sb[:, kc, :],
                            start=(kc == 0), stop=(kc == DKV // 128 - 1))
                    gtc = tt * (TW // 128) + tc4
                    nc.vector.tensor_copy(
                        v_sb[:, :, gtc, 0:128],
                        mm_ps.rearrange("p (h d) -> p h d", h=HL))
                # kr rope for this tile (all-bf16)
                tmp1b = tmpB.tile([128, TW], bf16, tag="tmp1b")
                rot_b = tmpB.tile([128, TW], bf16, tag="rotb")
                rope(kr_rope[:, ts], kr_sb, tmp1b, rot_b,
                     cos_sb[:, ts], sin_sb[:, ts])

        # ================= Phase C =================
        pC = ctx.enter_context(tc.tile_pool(name="persistC", bufs=1))
        aoT_sb = pC.tile([128, HL, T], bf16, tag="aoT")
        # wo prefetch (used in phase D)
        wo_sb = pC.tile([128, HL, D], bf16, tag="wo")
        nc.sync.dma_start(
            out=wo_sb, in_=wo_d.rearrange("(c p) m -> p c m", p=128))
        # static diagonal mask: mask[k, j] = NEG where j < k (query < key)
        mask_sb = pC.tile([128, 128], f32, tag="mask")
        nc.gpsimd.memset(mask_sb, 0.0)
        nc.gpsimd.affine_select(
            out=mask_sb, in_=mask_sb,
            compare_op=mybir.AluOpType.is_ge, fill=NEG,
            base=0, pattern=[[1, 128]], channel_multiplier=-1)

        with tc.tile_pool(name="ptC", bufs=3) as ptC, \
             tc.tile_pool(name="noC", bufs=2) as noC, \
             tc.tile_pool(name="rC", bufs=2) as rC, \
             tc.tile_pool(name="psS", bufs=2, space="PSUM") as psS, \
             tc.tile_pool(name="psPV", bufs=4, space="PSUM") as psPV, \
             tc.tile_pool(name="psTr", bufs=1, space="PSUM") as psTr:

            for h in range(HL):
                qr_part = slice((h % 2) * 64, (h % 2) * 64 + 64)
                jj = h // 2
                for c in range(NT):
                    q0 = c * TW
                    b0 = q0 // 128          # global index of first block
                    nkt = 4 * c + 4
                    pv_ps = [psPV.tile([128, 132], f32, tag="psPV",
                                       name=f"pv_{h}_{c}_{i}")
                             for i in range(4)]
                    for kt in range(nkt):
                        k0 = kt * 128
                        qs_lo = max(q0, k0)
                        w = q0 + TW - qs_lo
                        s_ps = psS.tile([128, TW], f32, tag="psS")
                        nc.tensor.matmul(
                            s_ps[:, 0:w], lhsT=kT_sb[:, h, k0:k0 + 128],
                            rhs=qT_sb[:, h, qs_lo:q0 + TW],
                            start=True, stop=False)
                        nc.tensor.matmul(
                            s_ps[:, 0:w], lhsT=kr_rope[qr_part, k0:k0 + 128],
                            rhs=qrT_sb[qr_part, jj, qs_lo:q0 + TW],
                            start=False, stop=True)
                        if k0 >= q0:  # diagonal block leads this row
                            nc.vector.tensor_add(
                                s_ps[:, 0:128], s_ps[:, 0:128], mask_sb)
                        pt_sb = ptC.tile([128, TW], bf16, tag="ptC")
                        nc.scalar.activation(
                            pt_sb[:, 0:w], s_ps[:, 0:w], func=AF.Exp,
                            scale=SCALE)
                        for g in range(max(b0, kt), b0 + 4):
                            rel = g - b0
                            off = g * 128 - qs_lo
                            nc.tensor.matmul(
                                pv_ps[rel][:, 0:129],
                                lhsT=pt_sb[:, off:off + 128],
                                rhs=v_sb[:, h, kt, 0:129],
                                start=(kt == 0), stop=(kt == g))
                            if kt == g:  # this query block is complete
                                r_sb = rC.tile([128, 1], f32, tag="rC")
                                nc.vector.reciprocal(
                                    r_sb, pv_ps[rel][:, 128:129])
                                no_sb = noC.tile([128, 128], bf16, tag="noC")
                                nc.vector.tensor_scalar_mul(
                                    no_sb, pv_ps[rel][:, 0:128], r_sb)
                                tr_ps = psTr.tile([128, 128], bf16, tag="psTr")
                                nc.tensor.transpose(tr_ps, no_sb, ident_bf)
                                nc.vector.tensor_copy(
                                    aoT_sb[:, h, g * 128:(g + 1) * 128], tr_ps)

        # ================= Phase D =================
        with tc.tile_pool(name="oD", bufs=3) as oD, \
             tc.tile_pool(name="psD", bufs=3, space="PSUM") as psD:
            for nt in range(NT):
                ns = slice(nt * TW, (nt + 1) * TW)
                for dc in range(D // 128):
                    o_ps = psD.tile([128, TW], f32, tag="psD")
                    for hc in range(HL):
                        nc.tensor.matmul(
                            o_ps,
                            lhsT=wo_sb[:, hc, dc * 128:(dc + 1) * 128],
                            rhs=aoT_sb[:, hc, ns],
                            start=(hc == 0), stop=(hc == HL - 1))
                    o_sb = oD.tile([128, TW], bf16, tag="oD")
                    nc.scalar.copy(o_sb, o_ps)
                    nc.sync.dma_start(
                        out=out_d[dc * 128:(dc + 1) * 128, ns], in_=o_sb)

    nc.finalize()
    return nc


def _rope_tables():
    inv_freq = (1.0 / (ROPE_BASE ** (np.arange(0, DR, 2, dtype=np.float32) / DR))
                ).astype(np.float32)
    t = np.arange(T, dtype=np.float32)
    freqs = np.outer(t, inv_freq).astype(np.float32)        # (T, 32)
    emb = np.concatenate([freqs, freqs], axis=-1)           # (T, 64)
    cos = np.cos(emb).astype(np.float32).T                  # (64, T)
    sin = np.sin(emb).astype(np.float32).T
    cos128 = np.ascontiguousarray(np.concatenate([cos, cos], 0))  # (128, T)
    sin128 = np.ascontiguousarray(np.concatenate([sin, sin], 0))
    return cos128, sin128


def kernel(x, W_DQ, W_UQ, W_QR, W_DKV, W_UK, W_UV, W_KR, W_O,
           q_norm_w, kv_norm_w):
    global LAST_EXEC_NS
    import ml_dtypes
    from concourse.bass_utils import run_bass_kernel_spmd

    bf = ml_dtypes.bfloat16
    x = np.asarray(x, dtype=np.float32)
    W_DQ = np.asarray(W_DQ, np.float32)
    W_UQ = np.asarray(W_UQ, np.float32)
    W_QR = np.asarray(W_QR, np.float32)
    W_DKV = np.asarray(W_DKV, np.float32)
    W_UK = np.asarray(W_UK, np.float32)
    W_UV = np.asarray(W_UV, np.float32)
    W_KR = np.asarray(W_KR, np.float32)
    W_O = np.asarray(W_O, np.float32)
    q_norm_w = np.asarray(q_norm_w, np.float32)
    kv_norm_w = np.asarray(kv_norm_w, np.float32)

    # fold norm weights into the up-projections (w==1 in practice)
    wuq_f = W_UQ * q_norm_w[:, None]
    wqr_f = W_QR * q_norm_w[:, None]
    wuk_f = W_UK * kv_norm_w[:, None]
    wuv_f = W_UV * kv_norm_w[:, None]

    wall = np.ascontiguousarray(
        np.concatenate([W_DQ, W_DKV, W_KR, W_KR], axis=1)).astype(bf)
    cos128, sin128 = _rope_tables()
    cos128 = cos128.astype(bf)
    sin128 = sin128.astype(bf)

    wuq_h = wuq_f.reshape(DQ, H, DH)
    wqr_h = wqr_f.reshape(DQ, H, DR)
    wuk_h = wuk_f.reshape(DKV, H, DH)
    wuv_h = wuv_f.reshape(DKV, H, DH)
    wo_h = W_O.reshape(H, DH, D)

    in_maps = []
    for ci in range(NCORES):
        b, hg = divmod(ci, H // HL)
        hsl = slice(hg * HL, (hg + 1) * HL)
        in_maps.append({
            "xt": np.ascontiguousarray(x[b].T).astype(bf),
            "wall": wall,
            "wuq": np.ascontiguousarray(
                wuq_h[:, hsl].reshape(DQ, HL * DH)).astype(bf),
            "wqr": np.ascontiguousarray(
                wqr_h[:, hsl].reshape(DQ, HL * DR)).astype(bf),
            "wuk": np.ascontiguousarray(
                wuk_h[:, hsl].reshape(DKV, HL * DH)).astype(bf),
            "wuv": np.ascontiguousarray(
                wuv_h[:, hsl].reshape(DKV, HL * DH)).astype(bf),
            "wo": np.ascontiguousarray(
                wo_h[hsl].reshape(HL * DH, D)).astype(bf),
            "costab": cos128,
            "sintab": sin128,
        })

    if "nc" not in _CACHE:
        _CACHE["nc"] = _build()
    nc = _CACHE["nc"]

    import os as _os
    _trace = _os.environ.get("MLA_TRACE") == "1"
    res = run_bass_kernel_spmd(
        nc, in_maps, core_ids=list(range(NCORES)), trace=_trace)
    LAST_EXEC_NS = res.exec_time_ns
    outs = [res.results[ci]["final_t"] for ci in range(NCORES)]

    out = np.zeros((B, T, D), np.float32)
    for ci in range(NCORES):
        b = ci // (H // HL)
        out[b] += outs[ci].T.astype(np.float32)
    return out


# revision 31
# speedup vs baseline: 1.6518x; 1.2307x over previous
"""Multi-Head Latent Attention (MLA) forward on 8 Trainium2 NeuronCores.

Problem shapes (hardcoded, self-contained):
  B=2, T=2048, D=2048, H=16, DH=128, DKV=512, DQ=1024, DR=64, fp32 I/O.

Sharding: core ci = b*4 + hg  (b in {0,1}, hg in {0..3}); each core owns one
batch element and 4 heads.  Up-projection weights sharded over heads; the
final W_O matmul is input-dim sharded, so each core emits a partial (D,T)
output which the host sums (in f32) over the 4 head-group cores per batch.

All device matmul operands are bf16 (PSUM accumulation stays f32); the host
pre-converts inputs, so there are no on-device rounding copies.  Dataflow is
feature-major (features on partitions, tokens free) so every contraction
lands on the partition dim with no transposes of activations.

Phases (single NEFF):
  A+B fused per 512-token tile: latents (cq | ckv | kr) from one x-tile pass
     (W_KR is packed twice in the last 128-column block so the duplicated
     kr halves come out of the same matmul), rmsnorm via ones-matmul sumsq
     + PE-broadcast of 1/rms, then immediately up-project to qT/qrT/kT/v
     while the latents are still in SBUF.  RoPE applied to qr/kr inline.
  C: per (head, 1024-query half): exact-width causal S^T blocks (keys on
     partitions), one static 128x128 diagonal mask, exp -> bf16 P^T, PV
     matmuls interleaved per key-block into a wide PSUM with a ones-column
     denominator, normalize + PE-transpose into feature-major aoT.
  D: final.T = W_O_shard.T @ aoT -> DRAM (D, T) bf16.
"""

import math

import numpy as np

B, T, D = 2, 2048, 2048
H, DH = 16, 128
DKV, DQ, DR = 512, 1024, 64
ROPE_BASE = 500000.0
EPS = 1e-6
SCALE = 1.0 / math.sqrt(DH + DR)

HL = 4            # heads per core
NCORES = 8
TW = 512          # token tile width for A+B
NT = T // TW      # 4 token tiles
NKC = D // 128    # 16 contraction chunks over D
MTOT = 1664       # latent columns: 1024 cq | 512 ckv | 64 kr | 64 kr (dup)
NMC = MTOT // 128  # 13 column blocks
NEG = -1.0e30

_CACHE: dict = {}
LAST_EXEC_NS = None


def _build():
    from contextlib import ExitStack

    import concourse.mybir as mybir
    import concourse.tile as tile
    from concourse.bacc import Bacc
    from concourse.masks import make_identity

    f32 = mybir.dt.float32
    bf16 = mybir.dt.bfloat16
    AF = mybir.ActivationFunctionType

    nc = Bacc("TRN2")

    xT_d = nc.dram_tensor("xt", (D, T), bf16, kind="ExternalInput")
    # wall: host-relaid to (128, NMC, NKC*128) so each 128-column latent block
    # is one contiguous full-bandwidth DMA (compute starts after block 0)
    wall_d = nc.dram_tensor("wall", (128, NMC, D), bf16, kind="ExternalInput")
    wuq_d = nc.dram_tensor("wuq", (DQ, HL * DH), bf16, kind="ExternalInput")
    wqr_d = nc.dram_tensor("wqr", (DQ, HL * DR), bf16, kind="ExternalInput")
    wuk_d = nc.dram_tensor("wuk", (DKV, HL * DH), bf16, kind="ExternalInput")
    wuv_d = nc.dram_tensor("wuv", (DKV, HL * DH), bf16, kind="ExternalInput")
    wo_d = nc.dram_tensor("wo", (HL * DH, D), bf16, kind="ExternalInput")
    cos_d = nc.dram_tensor("costab", (128, T), bf16, kind="ExternalInput")
    sin_d = nc.dram_tensor("sintab", (128, T), bf16, kind="ExternalInput")
    out_d = nc.dram_tensor("final_t", (D, T), bf16, kind="ExternalOutput")

    with tile.TileContext(nc) as tc, ExitStack() as ctx:
        persist = ctx.enter_context(tc.tile_pool(name="persist", bufs=1))

        # --- constants ---
        ones_sb = persist.tile([128, 128], bf16, tag="ones")
        nc.gpsimd.memset(ones_sb, 1.0)
        ident_sb = persist.tile([128, 128], f32, tag="identf")
        make_identity(nc, ident_sb)
        ident_bf = persist.tile([128, 128], bf16, tag="identbf")
        nc.vector.tensor_copy(ident_bf, ident_sb)
        eps_sb = persist.tile([1, 1], f32, tag="eps")
        nc.vector.memset(eps_sb, EPS)

        # --- persistent activations (consumed by phase C/D) ---
        qT_sb = persist.tile([128, HL, T], bf16, tag="qT")
        qrT_sb = persist.tile([128, HL // 2, T], bf16, tag="qrT")
        kT_sb = persist.tile([128, HL, T], bf16, tag="kT")
        v_sb = persist.tile([128, HL, T // 128, 132], bf16, tag="v")
        nc.vector.memset(v_sb[:, :, :, 128:129], 1.0)
        kr_rope = persist.tile([128, T], bf16, tag="kr_rope")

        def rope(dst, src, tmp1, rot_sb, cos_ap, sin_ap):
            # dst = src*cos + rotate_half(src)*sin over two 64-row halves
            for hh in range(2):
                lo = slice(hh * 64, hh * 64 + 32)
                hi = slice(hh * 64 + 32, hh * 64 + 64)
                nc.scalar.mul(rot_sb[lo, :], src[hi, :], -1.0)
                nc.scalar.copy(rot_sb[hi, :], src[lo, :])
            nc.vector.tensor_mul(tmp1, src, cos_ap)
            nc.vector.tensor_mul(rot_sb, rot_sb, sin_ap)
            nc.vector.tensor_add(dst, tmp1, rot_sb)

        # ================= Phase A+B (fused per token tile) =================
        with tc.tile_pool(name="wA", bufs=1) as wA, \
             tc.tile_pool(name="wB", bufs=1) as wB, \
             tc.tile_pool(name="trig", bufs=1) as trig, \
             tc.tile_pool(name="xA", bufs=2) as xA, \
             tc.tile_pool(name="cqP", bufs=1) as cqP, \
             tc.tile_pool(name="ckvP", bufs=1) as ckvP, \
             tc.tile_pool(name="krP", bufs=1) as krP, \
             tc.tile_pool(name="sqA", bufs=1) as sqA, \
             tc.tile_pool(name="nrmA", bufs=1) as nrmA, \
             tc.tile_pool(name="tmpB", bufs=1) as tmpB, \
             tc.tile_pool(name="psMM", bufs=5, space="PSUM") as psMM, \
             tc.tile_pool(name="psSum", bufs=1, space="PSUM") as psSum, \
             tc.tile_pool(name="psT", bufs=1, space="PSUM") as psT, \
             tc.tile_pool(name="psBC", bufs=1, space="PSUM") as psBC:

            # weight / trig loads (per-block DMAs so compute starts early);
            # first x tile is also chunked and issued before the weights
            # interleave x chunks with wall blocks so wall[mc] lands just
            # before the mc-th matmul group needs it
            x0_sb = xA.tile([128, NKC, TW], bf16, tag="xA")
            wall_sb = wA.tile([128, NMC, D], bf16, tag="wall")
            nc.sync.dma_start(out=x0_sb[:, 0, :], in_=xT_d[0:128, 0:TW])
            nc.sync.dma_start(out=wall_sb[:, 0, :], in_=wall_d[:, 0, :])
            next_mc = 1
            for kc in range(1, NKC):
                nc.sync.dma_start(
                    out=x0_sb[:, kc, :],
                    in_=xT_d[kc * 128:(kc + 1) * 128, 0:TW])
                if kc % 4 == 0 and next_mc < NMC:
                    nc.sync.dma_start(
                        out=wall_sb[:, next_mc, :], in_=wall_d[:, next_mc, :])
                    next_mc += 1
            for mc in range(next_mc, NMC):
                nc.sync.dma_start(out=wall_sb[:, mc, :], in_=wall_d[:, mc, :])
            wuq_sb = wB.tile([128, DQ // 128, HL * DH], bf16, tag="wuq")
            wqr_sb = wB.tile([128, DQ // 128, HL * DR], bf16, tag="wqr")
            wuk_sb = wB.tile([128, DKV // 128, HL * DH], bf16, tag="wuk")
            wuv_sb = wB.tile([128, DKV // 128, HL * DH], bf16, tag="wuv")
            nc.sync.dma_start(
                out=wuq_sb, in_=wuq_d.rearrange("(c p) m -> p c m", p=128))
            nc.sync.dma_start(
                out=wqr_sb, in_=wqr_d.rearrange("(c p) m -> p c m", p=128))
            nc.sync.dma_start(
                out=wuk_sb, in_=wuk_d.rearrange("(c p) m -> p c m", p=128))
            nc.sync.dma_start(
                out=wuv_sb, in_=wuv_d.rearrange("(c p) m -> p c m", p=128))
            cos_sb = trig.tile([128, T], bf16, tag="cos")
            sin_sb = trig.tile([128, T], bf16, tag="sin")
            nc.sync.dma_start(out=cos_sb, in_=cos_d[:, :])
            nc.sync.dma_start(out=sin_sb, in_=sin_d[:, :])

            for tt in range(NT):
                ts = slice(tt * TW, (tt + 1) * TW)
                if tt == 0:
                    x_sb = x0_sb
                else:
                    x_sb = xA.tile([128, NKC, TW], bf16, tag="xA")
                    for kc in range(NKC):
                        nc.sync.dma_start(
                            out=x_sb[:, kc, :],
                            in_=xT_d[kc * 128:(kc + 1) * 128, ts])

                cq_sb = cqP.tile([128, DQ // 128, TW], bf16, tag="cq")
                ckv_sb = ckvP.tile([128, DKV // 128, TW], bf16, tag="ckv")
                kr_sb = krP.tile([128, TW], bf16, tag="kr")

                # latent matmuls + rmsnorm statistics (squares accumulated on
                # DVE; one ones-matmul per group does the partition reduction)
                acc_cq = sqA.tile([128, TW], f32, tag="accCq")
                acc_kv = sqA.tile([128, TW], f32, tag="accKv")
                acc_cq_bf = sqA.tile([128, TW], bf16, tag="accCqBf")
                acc_kv_bf = sqA.tile([128, TW], bf16, tag="accKvBf")
                for mc in range(NMC):
                    mm_ps = psMM.tile([128, TW], f32, tag="psMM")
                    for kc in range(NKC):
                        nc.tensor.matmul(
                            mm_ps,
                            lhsT=wall_sb[:, mc, kc * 128:(kc + 1) * 128],
                            rhs=x_sb[:, kc, :],
                            start=(kc == 0), stop=(kc == NKC - 1))
                    if mc < 8:
                        dst = cq_sb[:, mc, :]
                    elif mc < 12:
                        dst = ckv_sb[:, mc - 8, :]
                    else:
                        dst = kr_sb
                    nc.scalar.copy(dst, mm_ps)
                    if mc < 12:
                        acc = acc_cq if mc < 8 else acc_kv
                        if mc % 8 == 0:
                            nc.scalar.square(acc, mm_ps)
                        else:
                            sq_sb = sqA.tile([128, TW], f32, tag="sqA")
                            nc.scalar.square(sq_sb, mm_ps)
                            if mc in (7, 11):  # last of group: emit bf16
                                out = acc_cq_bf if mc == 7 else acc_kv_bf
                                nc.vector.tensor_add(out, acc, sq_sb)
                            else:
                                nc.vector.tensor_add(acc, acc, sq_sb)

                # 1/rms factors; the per-token scale commutes through the
                # linear up-projections, so B consumes RAW latents and the
                # scale is applied on B's outputs.  The norm chain's PE ops
                # (sum reduce, broadcast, column transpose) are interleaved
                # between B matmul groups so PE never waits on ACT/DVE.
                # ---- B: up-projections (raw latents in, scale on outputs).
                # The norm chain's tiny PE ops (sum reduce, bc broadcast,
                # column transpose) are threaded between the UQ/QR matmul
                # groups so PE never waits on the ACT/DVE parts of the chain.
                def uq_group(h):
                    mm_ps = psMM.tile([128, TW], f32, tag="psMM")
                    for kc in range(DQ // 128):
                        nc.tensor.matmul(
                            mm_ps,
                            lhsT=wuq_sb[:, kc, h * 128:(h + 1) * 128],
                            rhs=cq_sb[:, kc, :],
                            start=(kc == 0), stop=(kc == DQ // 128 - 1))
                    return mm_ps

                def qr_group(j):
                    mm_ps = psMM.tile([128, TW], f32, tag="psMM")
                    for kc in range(DQ // 128):
                        nc.tensor.matmul(
                            mm_ps,
                            lhsT=wqr_sb[:, kc, j * 128:(j + 1) * 128],
                            rhs=cq_sb[:, kc, :],
                            start=(kc == 0), stop=(kc == DQ // 128 - 1))
                    return mm_ps

                nrm_bf = {}

                def norm_chain(key, acc, ndiv):
                    sum_ps = psSum.tile([1, TW], f32, tag="psSum")
                    nc.tensor.matmul(
                        sum_ps, lhsT=ones_sb[:, 0:1], rhs=acc,
                        start=True, stop=True)
                    nrm_f = nrmA.tile([1, TW], f32, tag="nrmA")
                    nc.scalar.activation(
                        nrm_f, sum_ps, func=AF.Sqrt, bias=eps_sb,
                        scale=1.0 / ndiv)
                    nc.vector.reciprocal(nrm_f, nrm_f)
                    nbf = nrmA.tile([1, TW], bf16, tag=f"nrmBf{key}")
                    nc.vector.tensor_copy(nbf, nrm_f)
                    nrm_bf[key] = nbf

                bc = {}

                def bc_chain(key):
                    bc_ps = psBC.tile([128, TW], f32, tag="psBC")
                    nc.tensor.matmul(
                        bc_ps, lhsT=ones_sb[0:1, :], rhs=nrm_bf[key],
                        start=True, stop=True)
                    bc_sb = nrmA.tile([128, TW], bf16, tag=f"bcSb{key}")
                    nc.vector.tensor_copy(bc_sb, bc_ps)
                    bc[key] = bc_sb

                uq_ps = [uq_group(0), uq_group(1)]
                norm_chain("q", acc_cq_bf, DQ)
                norm_chain("kv", acc_kv_bf, DKV)
                uq_ps += [uq_group(2), uq_group(3)]
                bc_chain("q")
                qr0_ps = qr_group(0)
                bc_chain("kv")
                bcT = {}
                t_ps = psT.tile([128, 4], f32, tag="psT")
                for tc4 in range(TW // 128):
                    nc.tensor.matmul(
                        t_ps[:, tc4:tc4 + 1],
                        lhsT=nrm_bf["kv"][0:1, tc4 * 128:(tc4 + 1) * 128],
                        rhs=ones_sb[0:1, 0:1],
                        start=True, stop=True)
                for tc4 in range(TW // 128):
                    tv = nrmA.tile([128, 1], f32, tag=f"bcT{tc4}")
                    nc.vector.tensor_copy(tv, t_ps[:, tc4:tc4 + 1])
                    bcT[tc4] = tv
                for h in range(HL):
                    nc.vector.tensor_mul(qT_sb[:, h, ts], uq_ps[h], bc["q"])
                qr_ps = [qr0_ps, qr_group(1)]
                for j in range(HL // 2):
                    tmp1 = tmpB.tile([128, TW], f32, tag="tmp1")
                    rot_sb = tmpB.tile([128, TW], f32, tag="rot")
                    rope(tmp1, qr_ps[j], tmp1, rot_sb,
                         cos_sb[:, ts], sin_sb[:, ts])
                    nc.vector.tensor_mul(qrT_sb[:, j, ts], tmp1, bc["q"])
                for h in range(HL):
                    mm_ps = psMM.tile([128, TW], f32, tag="psMM")
                    for kc in range(DKV // 128):
                        nc.tensor.matmul(
                            mm_ps,
                            lhsT=wuk_sb[:, kc, h * 128:(h + 1) * 128],
                            rhs=ckv_sb[:, kc, :],
                            start=(kc == 0), stop=(kc == DKV // 128 - 1))
                    nc.vector.tensor_mul(kT_sb[:, h, ts], mm_ps, bc["kv"])
                for tc4 in range(TW // 128):
                    mm_ps = psMM.tile([128, TW], f32, tag="psMM")
                    for kc in range(DKV // 128):
                        nc.tensor.matmul(
                            mm_ps,
                            lhsT=ckv_sb[:, kc, tc4 * 128:(tc4 + 1) * 128],
                            rhs=wuv_sb[:, kc, :],
                            start=(kc == 0), stop=(kc == DKV // 128 - 1))
                    gtc = tt * (TW // 128) + tc4
                    nc.vector.tensor_scalar_mul(
                        v_sb[:, :, gtc, 0:128],
                        mm_ps.rearrange("p (h d) -> p h d", h=HL),
                        bcT[tc4])
                # kr rope for this tile (all-bf16, no rms norm on k_R)
                tmp1b = tmpB.tile([128, TW], bf16, tag="tmp1b")
                rot_b = tmpB.tile([128, TW], bf16, tag="rotb")
                rope(kr_rope[:, ts], kr_sb, tmp1b, rot_b,
                     cos_sb[:, ts], sin_sb[:, ts])

        # ================= Phase C =================
        pC = ctx.enter_context(tc.tile_pool(name="persistC", bufs=1))
        aoT_sb = pC.tile([128, HL, T], bf16, tag="aoT")
        # wo prefetch (used in phase D)
        wo_sb = pC.tile([128, HL, D], bf16, tag="wo")
        nc.sync.dma_start(
            out=wo_sb, in_=wo_d.rearrange("(c p) m -> p c m", p=128))
        # static diagonal mask: mask[k, j] = NEG where j < k (query < key)
        mask_sb = pC.tile([128, 128], f32, tag="mask")
        nc.gpsimd.memset(mask_sb, 0.0)
        nc.gpsimd.affine_select(
            out=mask_sb, in_=mask_sb,
            compare_op=mybir.AluOpType.is_ge, fill=NEG,
            base=0, pattern=[[1, 128]], channel_multiplier=-1)

        with tc.tile_pool(name="ptC", bufs=5) as ptC, \
             tc.tile_pool(name="noC", bufs=3) as noC, \
             tc.tile_pool(name="rC", bufs=2) as rC, \
             tc.tile_pool(name="psS", bufs=3, space="PSUM") as psS, \
             tc.tile_pool(name="psPV", bufs=4, space="PSUM") as psPV, \
             tc.tile_pool(name="psTr", bufs=1, space="PSUM") as psTr:

            # (no_sb, h, g) blocks whose PE transpose is deferred one step so
            # the in-order PE never waits on the DVE normalize chain
            pending = []

            def flush_pending():
                no_sb, fh, fg = pending.pop(0)
                tr_ps = psTr.tile([128, 128], bf16, tag="psTr")
                nc.tensor.transpose(tr_ps, no_sb, ident_bf)
                nc.vector.tensor_copy(
                    aoT_sb[:, fh, fg * 128:(fg + 1) * 128], tr_ps)

            # big c-tiles first per head; the four tiny c=0 tiles run
            # back-to-back at the end so their exp latency hides across heads
            order = [(h, c) for h in range(HL) for c in range(NT - 1, 0, -1)]
            order += [(h, 0) for h in range(HL)]
            for h, c in order:
                qr_part = slice((h % 2) * 64, (h % 2) * 64 + 64)
                jj = h // 2
                if True:
                    q0 = c * TW
                    b0 = q0 // 128          # global index of first block
                    nkt = 4 * c + 4
                    pv_ps = [psPV.tile([128, 132], f32, tag="psPV",
                                       name=f"pv_{h}_{c}_{i}")
                             for i in range(4)]

                    def emit_pv(kt, pt_sb, h=h, q0=q0, b0=b0, pv_ps=pv_ps):
                        qs_lo = max(q0, kt * 128)
                        for g in range(max(b0, kt), b0 + 4):
                            rel = g - b0
                            off = g * 128 - qs_lo
                            nc.tensor.matmul(
                                pv_ps[rel][:, 0:129],
                                lhsT=pt_sb[:, off:off + 128],
                                rhs=v_sb[:, h, kt, 0:129],
                                start=(kt == 0), stop=(kt == g))
                            if kt == g:  # this query block is complete
                                if pending:
                                    flush_pending()
                                r_sb = rC.tile([128, 1], f32, tag="rC")
                                nc.vector.reciprocal(
                                    r_sb, pv_ps[rel][:, 128:129])
                                no_sb = noC.tile([128, 128], bf16, tag="noC")
                                nc.vector.tensor_scalar_mul(
                                    no_sb, pv_ps[rel][:, 0:128], r_sb)
                                pending.append((no_sb, h, g))

                    inflight = []
                    for kt in range(nkt):
                        k0 = kt * 128
                        qs_lo = max(q0, k0)
                        w = q0 + TW - qs_lo
                        s_ps = psS.tile([128, TW], f32, tag="psS")
                        nc.tensor.matmul(
                            s_ps[:, 0:w], lhsT=kT_sb[:, h, k0:k0 + 128],
                            rhs=qT_sb[:, h, qs_lo:q0 + TW],
                            start=True, stop=False)
                        nc.tensor.matmul(
                            s_ps[:, 0:w], lhsT=kr_rope[qr_part, k0:k0 + 128],
                            rhs=qrT_sb[qr_part, jj, qs_lo:q0 + TW],
                            start=False, stop=True)
                        if k0 >= q0:  # diagonal block leads this row
                            nc.vector.tensor_add(
                                s_ps[:, 0:128], s_ps[:, 0:128], mask_sb)
                        pt_sb = ptC.tile([128, TW], bf16, tag="ptC")
                        nc.scalar.activation(
                            pt_sb[:, 0:w], s_ps[:, 0:w], func=AF.Exp,
                            scale=SCALE)
                        inflight.append((kt, pt_sb))
                        if len(inflight) > 2:  # PV trails S/exp by 2 blocks
                            emit_pv(*inflight.pop(0))
                    for item in inflight:
                        emit_pv(*item)
            while pending:
                flush_pending()

        # ================= Phase D =================
        # out DMAs batched 4 dc-blocks at a time: each dma_start costs ~1us
        # of SP sequencer dispatch, and 64 of them saturate it
        with tc.tile_pool(name="oD", bufs=3) as oD, \
             tc.tile_pool(name="psD", bufs=6, space="PSUM") as psD:
            for nt in range(NT):
                ns = slice(nt * TW, (nt + 1) * TW)
                for dc0 in range(0, D // 128, 4):
                    o_sb = oD.tile([128, 4, TW], bf16, tag="oD")
                    for dj in range(4):
                        dc = dc0 + dj
                        o_ps = psD.tile([128, TW], f32, tag="psD")
                        for hc in range(HL):
                            nc.tensor.matmul(
                                o_ps,
                                lhsT=wo_sb[:, hc, dc * 128:(dc + 1) * 128],
                                rhs=aoT_sb[:, hc, ns],
                                start=(hc == 0), stop=(hc == HL - 1))
                        nc.scalar.copy(o_sb[:, dj, :], o_ps)
                    nc.sync.dma_start(
                        out=out_d[dc0 * 128:(dc0 + 4) * 128, ns].rearrange(
                            "(c p) t -> p c t", p=128),
                        in_=o_sb)

    nc.finalize()
    return nc


def _rope_tables():
    inv_freq = (1.0 / (ROPE_BASE ** (np.arange(0, DR, 2, dtype=np.float32) / DR))
                ).astype(np.float32)
    t = np.arange(T, dtype=np.float32)
    freqs = np.outer(t, inv_freq).astype(np.float32)        # (T, 32)
    emb = np.concatenate([freqs, freqs], axis=-1)           # (T, 64)
    cos = np.cos(emb).astype(np.float32).T                  # (64, T)
    sin = np.sin(emb).astype(np.float32).T
    cos128 = np.ascontiguousarray(np.concatenate([cos, cos], 0))  # (128, T)
    sin128 = np.ascontiguousarray(np.concatenate([sin, sin], 0))
    return cos128, sin128


def kernel(x, W_DQ, W_UQ, W_QR, W_DKV, W_UK, W_UV, W_KR, W_O,
           q_norm_w, kv_norm_w):
    global LAST_EXEC_NS
    import ml_dtypes
    from concourse.bass_utils import run_bass_kernel_spmd

    bf = ml_dtypes.bfloat16
    x = np.asarray(x, dtype=np.float32)
    W_DQ = np.asarray(W_DQ, np.float32)
    W_UQ = np.asarray(W_UQ, np.float32)
    W_QR = np.asarray(W_QR, np.float32)
    W_DKV = np.asarray(W_DKV, np.float32)
    W_UK = np.asarray(W_UK, np.float32)
    W_UV = np.asarray(W_UV, np.float32)
    W_KR = np.asarray(W_KR, np.float32)
    W_O = np.asarray(W_O, np.float32)
    q_norm_w = np.asarray(q_norm_w, np.float32)
    kv_norm_w = np.asarray(kv_norm_w, np.float32)

    # fold norm weights into the up-projections (w==1 in practice)
    wuq_f = W_UQ * q_norm_w[:, None]
    wqr_f = W_QR * q_norm_w[:, None]
    wuk_f = W_UK * kv_norm_w[:, None]
    wuv_f = W_UV * kv_norm_w[:, None]

    wall = np.concatenate([W_DQ, W_DKV, W_KR, W_KR], axis=1)
    # (D, MTOT) -> (128, NMC, D): partition-major, latent-block, contraction
    wall = np.ascontiguousarray(
        wall.reshape(NKC, 128, NMC, 128).transpose(1, 2, 0, 3)
        .reshape(128, NMC, D)).astype(bf)
    cos128, sin128 = _rope_tables()
    cos128 = cos128.astype(bf)
    sin128 = sin128.astype(bf)

    wuq_h = wuq_f.reshape(DQ, H, DH)
    wqr_h = wqr_f.reshape(DQ, H, DR)
    wuk_h = wuk_f.reshape(DKV, H, DH)
    wuv_h = wuv_f.reshape(DKV, H, DH)
    wo_h = W_O.reshape(H, DH, D)

    in_maps = []
    for ci in range(NCORES):
        b, hg = divmod(ci, H // HL)
        hsl = slice(hg * HL, (hg + 1) * HL)
        in_maps.append({
            "xt": np.ascontiguousarray(x[b].T).astype(bf),
            "wall": wall,
            "wuq": np.ascontiguousarray(
                wuq_h[:, hsl].reshape(DQ, HL * DH)).astype(bf),
            "wqr": np.ascontiguousarray(
                wqr_h[:, hsl].reshape(DQ, HL * DR)).astype(bf),
            "wuk": np.ascontiguousarray(
                wuk_h[:, hsl].reshape(DKV, HL * DH)).astype(bf),
            "wuv": np.ascontiguousarray(
                wuv_h[:, hsl].reshape(DKV, HL * DH)).astype(bf),
            "wo": np.ascontiguousarray(
                wo_h[hsl].reshape(HL * DH, D)).astype(bf),
            "costab": cos128,
            "sintab": sin128,
        })

    if "nc" not in _CACHE:
        _CACHE["nc"] = _build()
    nc = _CACHE["nc"]

    import os as _os
    _trace = _os.environ.get("MLA_TRACE") == "1"
    res = run_bass_kernel_spmd(
        nc, in_maps, core_ids=list(range(NCORES)), trace=_trace)
    LAST_EXEC_NS = res.exec_time_ns
    outs = [res.results[ci]["final_t"] for ci in range(NCORES)]

    out = np.zeros((B, T, D), np.float32)
    for ci in range(NCORES):
        b = ci // (H // HL)
        out[b] += outs[ci].T.astype(np.float32)
    return out
